# revision 27
# baseline (speedup 1.0000x reference)
"""2-layer GCN (GCNConv x2 + linear) on 8 Trainium2 NeuronCores.

Full-device implementation, node-sharded across cores (12,500 dst nodes each):

  * transform  z = x @ W (PE matmuls; x shipped pre-transposed in bf16)
  * z' = dinv*z allgathered across cores (on-device AllGather collective)
  * aggregation per 64-dst scatter block: gather z'[src] rows via indirect
    DMA (128 edges / instruction), build ewn-scaled one-hot selection
    matrices from per-edge dst offsets (DVE iota+is_equal), and accumulate
    messages into PSUM with TensorE matmuls (out^T layout [feat, dst], which
    makes the next layer's matmul need no transposes).  Self-loops are
    ordinary edges with weight dinv[d]; biases are rank-1 matmuls in PSUM.

Host side does cheap O(E) integer prep: degree/norm computation, a
serpentine-dealt node permutation per core that balances edge load across
the 196 scatter bins (drops gather tiles per bin to the minimum), one
argsort of edges by (core, bin), and per-core array packing.  The x
transfer (~100 MB bf16, transposed+permuted) is started asynchronously
while edge prep continues.
"""
import numpy as np
import ml_dtypes
from contextlib import ExitStack

N = 100000
E = 3200000
IN_DIM = 512
HID = 64
OUT = 5
C = 8          # cores
P = 128
NPC = N // C   # real nodes per core (12500)
NB = 98        # 128-row transform blocks per core
SHARD = NB * P          # padded rows per core (12544)
NTOT = C * SHARD        # allgathered table rows (100352)
SB = 64                 # scatter block width (dst nodes)
NSB = SHARD // SB       # scatter blocks per core (196)

BF16 = ml_dtypes.bfloat16

_prog_cache = {}


def _build_program(TPB, for_sim=False, no_collective=False, static_gather=False):
    """One SPMD program for all 8 cores; TPB = gather tiles per scatter block.

    for_sim=True replaces the collectives with equivalent-volume local DMA
    copies so the single-core cost simulator (TimelineSim) can run it.
    """
    from concourse import bacc, bass, tile, mybir

    NT = NSB * TPB  # gather tiles per core per layer

    nc = bacc.Bacc("TRN2", target_bir_lowering=False, debug=False,
                   num_devices=(1 if for_sim else C))
    dt = mybir.dt
    xt_in = nc.declare_dram_parameter("xt", [P, NB * 4 * P], dt.bfloat16, isOutput=False)
    w1_in = nc.declare_dram_parameter("w1", [P, 4 * HID], dt.bfloat16, isOutput=False)
    w2_in = nc.declare_dram_parameter("w2", [HID, HID], dt.bfloat16, isOutput=False)
    wf_in = nc.declare_dram_parameter("wf", [HID, OUT], dt.bfloat16, isOutput=False)
    b1_in = nc.declare_dram_parameter("b1", [1, HID], dt.bfloat16, isOutput=False)
    b2_in = nc.declare_dram_parameter("b2", [1, HID], dt.bfloat16, isOutput=False)
    bf_in = nc.declare_dram_parameter("bf", [1, OUT], dt.bfloat16, isOutput=False)
    dv1_in = nc.declare_dram_parameter("dv1", [P, NB], dt.float32, isOutput=False)
    src_in = nc.declare_dram_parameter("srci", [P, NT], dt.int32, isOutput=False)
    dst_in = nc.declare_dram_parameter("dsti", [P, NT], dt.int16, isOutput=False)
    ewn_in = nc.declare_dram_parameter("ewn", [P, NT], dt.bfloat16, isOutput=False)
    out_o = nc.declare_dram_parameter("out", [SHARD, OUT], dt.float32, isOutput=True)

    with tile.TileContext(nc) as tc:
        with ExitStack() as ctx:
            cpool = ctx.enter_context(tc.tile_pool(name="cpool", bufs=1))
            work = ctx.enter_context(tc.tile_pool(name="work", bufs=3))
            gpool = ctx.enter_context(tc.tile_pool(name="gpool", bufs=40))
            psum = ctx.enter_context(tc.tile_pool(name="psum", bufs=2, space="PSUM"))
            psag = ctx.enter_context(tc.tile_pool(name="psag", bufs=4, space="PSUM"))
            dram = ctx.enter_context(tc.tile_pool(name="dram", bufs=1, space="DRAM"))

            # ---- persistent constants ----
            w1_sb = cpool.tile([P, 4 * HID], dt.bfloat16)
            nc.sync.dma_start(out=w1_sb[:], in_=w1_in[:, :])
            w2_sb = cpool.tile([HID, HID], dt.bfloat16)
            nc.sync.dma_start(out=w2_sb[:], in_=w2_in[:, :])
            wf_sb = cpool.tile([HID, OUT], dt.bfloat16)
            nc.sync.dma_start(out=wf_sb[:], in_=wf_in[:, :])
            b1_sb = cpool.tile([1, HID], dt.bfloat16)
            nc.sync.dma_start(out=b1_sb[:], in_=b1_in[:, :])
            b2_sb = cpool.tile([1, HID], dt.bfloat16)
            nc.sync.dma_start(out=b2_sb[:], in_=b2_in[:, :])
            bf_sb = cpool.tile([1, OUT], dt.bfloat16)
            nc.sync.dma_start(out=bf_sb[:], in_=bf_in[:, :])
            dv1_sb = cpool.tile([P, NB], dt.float32)
            nc.sync.dma_start(out=dv1_sb[:], in_=dv1_in[:, :])
            src_sb = cpool.tile([P, NT], dt.int32)
            nc.sync.dma_start(out=src_sb[:], in_=src_in[:, :])
            dst_sb = cpool.tile([P, NT], dt.int16)
            nc.sync.dma_start(out=dst_sb[:], in_=dst_in[:, :])
            ewn_sb = cpool.tile([P, NT], dt.bfloat16)
            nc.sync.dma_start(out=ewn_sb[:], in_=ewn_in[:, :])

            iota_sb = cpool.tile([P, TPB * SB], dt.int16)
            nc.gpsimd.iota(iota_sb[:], pattern=[[0, TPB], [1, SB]], base=0,
                           channel_multiplier=0)
            ones_sb = cpool.tile([1, P], dt.bfloat16)
            nc.gpsimd.memset(ones_sb[:], 1.0)

            adsp = "Local" if (for_sim or no_collective) else "Shared"
            z1sh_d = dram.tile([SHARD, HID], dt.bfloat16)
            z1f_d = dram.tile([NTOT, HID], dt.bfloat16, addr_space=adsp)
            z2sh_d = dram.tile([SHARD, HID], dt.bfloat16)
            z2f_d = dram.tile([NTOT, HID], dt.bfloat16, addr_space=adsp)

            Relu = mybir.ActivationFunctionType.Relu
            Copy = mybir.ActivationFunctionType.Copy

            def transform1():
                for b in range(NB):
                    xtb = work.tile([P, 4 * P], dt.bfloat16, tag="xtb")
                    nc.sync.dma_start(out=xtb[:],
                                      in_=xt_in[:, b * 4 * P:(b + 1) * 4 * P])
                    zp = psum.tile([P, HID], dt.float32, space="PSUM", tag="zp")
                    for k in range(4):
                        nc.tensor.matmul(
                            out=zp[:],
                            lhsT=xtb[:, k * P:(k + 1) * P],
                            rhs=w1_sb[:, k * HID:(k + 1) * HID],
                            start=(k == 0), stop=(k == 3),
                        )
                    zprime = work.tile([P, HID], dt.bfloat16, tag="zprime")
                    nc.scalar.activation(out=zprime[:], in_=zp[:], func=Copy,
                                         scale=dv1_sb[:, b:b + 1])
                    nc.sync.dma_start(out=z1sh_d[b * P:(b + 1) * P, :], in_=zprime[:])

            def transform2(h1t_sb):
                for b in range(NB):
                    zp = psum.tile([P, HID], dt.float32, space="PSUM", tag="zp")
                    nc.tensor.matmul(out=zp[:],
                                     lhsT=h1t_sb[:, b * P:(b + 1) * P],
                                     rhs=w2_sb[:, :], start=True, stop=True)
                    zprime = work.tile([P, HID], dt.bfloat16, tag="zprime")
                    nc.scalar.activation(out=zprime[:], in_=zp[:], func=Copy,
                                         scale=dv1_sb[:, b:b + 1])
                    nc.sync.dma_start(out=z2sh_d[b * P:(b + 1) * P, :], in_=zprime[:])

            def allgather(sh_d, f_d):
                if for_sim or no_collective:
                    # equivalent-volume local traffic for the cost simulator
                    for c in range(C):
                        nc.sync.dma_start(
                            out=f_d[c * SHARD:(c + 1) * SHARD, :], in_=sh_d[:, :])
                    return
                nc.gpsimd.collective_compute(
                    "AllGather", mybir.AluOpType.bypass,
                    replica_groups=[list(range(C))],
                    ins=[sh_d[:]], outs=[f_d[:]],
                )

            def aggregate(zf_d, brow_sb, ht_sb):
                """ht[f, d] = relu of sum_e ewn*z'[src] + b (self-loops are edges).

                Gathers are issued one block ahead of their consuming matmuls
                so the GPSIMD SWDGE queue (the serial resource) never idles.
                """
                def issue_gathers(b):
                    gts = []
                    for t in range(TPB):
                        gt = gpool.tile([P, HID], dt.bfloat16, tag="g")
                        ti = b * TPB + t
                        if static_gather:
                            ro = (ti % 700) * P
                            nc.sync.dma_start(out=gt[:], in_=zf_d[ro:ro + P, :])
                        else:
                            nc.gpsimd.indirect_dma_start(
                                out=gt[:],
                                out_offset=None,
                                in_=zf_d[:, :],
                                in_offset=bass.IndirectOffsetOnAxis(
                                    ap=src_sb[:, ti:ti + 1], axis=0),
                            )
                        gts.append(gt)
                    return gts

                gcur = issue_gathers(0)
                for b in range(NSB):
                    gnext = issue_gathers(b + 1) if b + 1 < NSB else None
                    oh = work.tile([P, TPB * SB], dt.bfloat16, tag="oh")
                    nc.vector.tensor_tensor(
                        out=oh[:].rearrange('p (t j) -> p t j', t=TPB),
                        in0=iota_sb[:].rearrange('p (t j) -> p t j', t=TPB),
                        in1=dst_sb[:, b * TPB:(b + 1) * TPB, None].to_broadcast(
                            [P, TPB, SB]),
                        op=mybir.AluOpType.is_equal,
                    )
                    nc.vector.tensor_tensor(
                        out=oh[:].rearrange('p (t j) -> p t j', t=TPB),
                        in0=oh[:].rearrange('p (t j) -> p t j', t=TPB),
                        in1=ewn_sb[:, b * TPB:(b + 1) * TPB, None].to_broadcast(
                            [P, TPB, SB]),
                        op=mybir.AluOpType.mult,
                    )
                    acc = psag.tile([HID, SB], dt.float32, space="PSUM", tag="acc")
                    for t in range(TPB):
                        nc.tensor.matmul(
                            out=acc[:], lhsT=gcur[t][:],
                            rhs=oh[:, t * SB:(t + 1) * SB],
                            start=(t == 0), stop=False,
                        )
                    # bias: acc[f, d] += b[f] * 1
                    nc.tensor.matmul(
                        out=acc[:], lhsT=brow_sb[0:1, :],
                        rhs=ones_sb[0:1, 0:SB], start=False, stop=True,
                    )
                    nc.scalar.activation(out=ht_sb[:, b * SB:(b + 1) * SB],
                                         in_=acc[:], func=Relu)
                    gcur = gnext

            def final(h2t_sb):
                for b in range(NB):
                    fp = psum.tile([P, OUT], dt.float32, space="PSUM", tag="fp")
                    nc.tensor.matmul(out=fp[:],
                                     lhsT=h2t_sb[:, b * P:(b + 1) * P],
                                     rhs=wf_sb[:, :], start=True, stop=False)
                    nc.tensor.matmul(out=fp[:], lhsT=ones_sb[0:1, 0:P],
                                     rhs=bf_sb[0:1, :], start=False, stop=True)
                    o_sb = work.tile([P, OUT], dt.float32, tag="osb")
                    nc.scalar.activation(out=o_sb[:], in_=fp[:], func=Copy)
                    nc.sync.dma_start(out=out_o[b * P:(b + 1) * P, :], in_=o_sb[:])

            transform1()
            allgather(z1sh_d, z1f_d)
            h1t_sb = cpool.tile([HID, SHARD], dt.bfloat16)
            aggregate(z1f_d, b1_sb, h1t_sb)
            transform2(h1t_sb)
            allgather(z2sh_d, z2f_d)
            h2t_sb = cpool.tile([HID, SHARD], dt.bfloat16)
            aggregate(z2f_d, b2_sb, h2t_sb)
            final(h2t_sb)

    nc.compile()
    return nc


def _host_prep(x, edge_index, edge_attr, W1, b1, W2, b2, Wf, bf,
               on_xt=None):
    src0 = np.asarray(edge_index[0], dtype=np.int64)
    dst0 = np.asarray(edge_index[1], dtype=np.int64)
    ew = np.asarray(edge_attr, dtype=np.float32)

    deg = np.bincount(dst0, weights=ew, minlength=N).astype(np.float32) + 1.0
    dinv = (1.0 / np.sqrt(deg)).astype(np.float32)

    # self-loops become ordinary edges: msg = dinv[d] * z'[d]
    allnodes = np.arange(N, dtype=np.int64)
    src = np.concatenate([src0, allnodes])
    dst = np.concatenate([dst0, allnodes])
    ewn = np.concatenate([(ew * dinv[dst0]).astype(np.float32), dinv])
    EE = E + N

    # per-core node permutation: serpentine-deal nodes (by degree desc) into
    # the 196 scatter bins so every bin has near-equal edge load -> TPB drops.
    degi = np.bincount(dst0, minlength=N).astype(np.int64) + 1
    rowof = np.empty((C, SHARD), dtype=np.int32)   # node-rel -> shard row
    for c in range(C):
        dg = np.zeros(SHARD, dtype=np.int64)
        dg[:NPC] = degi[c * NPC:(c + 1) * NPC]
        idx = np.argsort(-dg, kind="stable")
        rows = idx.reshape(SHARD // NSB, NSB).copy()
        rows[1::2] = rows[1::2, ::-1]
        # node at rows[i, b] -> shard row b*64 + i
        rr = np.empty(SHARD, dtype=np.int32)
        pos = (np.arange(NSB)[None, :] * SB
               + np.arange(SHARD // NSB)[:, None]).astype(np.int32)
        rr[rows.ravel()] = pos.ravel()
        rowof[c] = rr
    rowof_g = np.empty(N, dtype=np.int32)   # global node id -> table row
    for c in range(C):
        rowof_g[c * NPC:(c + 1) * NPC] = rowof[c, :NPC] + c * SHARD

    # x block-major transposed: XTc[c][p, b*512 + k*128 + m] = x[row b*128+m, k*128+p]
    xb = np.asarray(x, dtype=np.float32).astype(BF16)
    xpad = np.zeros((C, SHARD, IN_DIM), dtype=BF16)
    xpad[np.arange(C)[:, None], rowof[:, :NPC]] = xb.reshape(C, NPC, IN_DIM)
    XTc = np.ascontiguousarray(
        xpad.reshape(C, NB, P, 4, P).transpose(0, 4, 1, 3, 2)
    ).reshape(C, P, NB * 4 * P)
    if on_xt is not None:
        on_xt(XTc)   # lets the caller start the device transfer early

    core = (dst // NPC).astype(np.int32)
    nrel = (dst - core.astype(np.int64) * NPC).astype(np.int64)
    drel = rowof[core, nrel].astype(np.int64)      # permuted shard row of dst
    gblk = (core * NSB + drel // SB).astype(np.int64)  # global scatter block
    dst16 = (drel % SB).astype(np.int16)

    order = np.argsort(gblk, kind="stable")
    gsrt = gblk[order]
    cnt = np.bincount(gblk, minlength=C * NSB)
    TPB = int(np.ceil(cnt.max() / P))
    spb = TPB * P
    bstart = np.concatenate(([0], np.cumsum(cnt)))
    pos = np.arange(EE, dtype=np.int64) - bstart[gsrt]
    slot = gsrt.astype(np.int64) * spb + pos   # global slot across cores

    gidx = rowof_g[src].astype(np.int32)   # permuted row in allgathered table

    NTS = C * NSB * spb
    SRC = np.empty(NTS, dtype=np.int32)
    SRC[:] = np.resize(np.arange(SHARD, dtype=np.int32), NTS)  # spread pads
    DST = np.full(NTS, SB, dtype=np.int16)    # pad dstrel -> no one-hot match
    EWN = np.zeros(NTS, dtype=np.float32)
    SRC[slot] = gidx[order]
    DST[slot] = dst16[order]
    EWN[slot] = ewn[order]

    # device layout [C, 128, NT]: slot -> (blk, tile, p); SBUF col = blk*TPB + t
    def dev_layout(a):
        a = a.reshape(C, NSB * TPB, P)        # [(c), (blk*tile), p]
        return np.ascontiguousarray(a.transpose(0, 2, 1))

    SRCd = dev_layout(SRC)
    DSTd = dev_layout(DST)
    EWNd = dev_layout(EWN.astype(BF16))

    # dinv packs [C, 128, NB] (rows permuted like the shard)
    dpad_v = np.zeros((C, SHARD), dtype=np.float32)
    ar = np.arange(C)[:, None]
    dpad_v[ar, rowof[:, :NPC]] = dinv.reshape(C, NPC)
    DV1 = np.ascontiguousarray(dpad_v.reshape(C, NB, P).transpose(0, 2, 1))

    W1p = np.ascontiguousarray(
        np.asarray(W1, dtype=np.float32).astype(BF16)
        .reshape(4, P, HID).transpose(1, 0, 2).reshape(P, 4 * HID))
    W2p = np.asarray(W2, dtype=np.float32).astype(BF16)
    Wfp = np.asarray(Wf, dtype=np.float32).astype(BF16)
    b1p = np.asarray(b1, dtype=np.float32).astype(BF16).reshape(1, HID)
    b2p = np.asarray(b2, dtype=np.float32).astype(BF16).reshape(1, HID)
    bfp = np.asarray(bf, dtype=np.float32).astype(BF16).reshape(1, OUT)

    in_maps = []
    for c in range(C):
        in_maps.append(dict(
            xt=XTc[c], w1=W1p, w2=W2p, wf=Wfp, b1=b1p, b2=b2p, bf=bfp,
            dv1=DV1[c], srci=SRCd[c], dsti=DSTd[c], ewn=EWNd[c],
        ))
    return in_maps, TPB, rowof


def _install_neff_cache():
    """Content-addressed /tmp cache around the walrus compile (the slow step)."""
    import hashlib
    import os
    import shutil
    from concourse import bass2jax as b2j

    if getattr(b2j, "_neff_disk_cache", False):
        return
    orig = b2j.compile_bir_kernel
    cache_dir = "/tmp/bass_neff_cache"
    os.makedirs(cache_dir, exist_ok=True)

    def cached(bir_json, tmpdir, neff_name="file.neff"):
        key = hashlib.sha256(bir_json).hexdigest()
        hit = os.path.join(cache_dir, key + ".neff")
        if os.path.exists(hit):
            dst = os.path.join(tmpdir, neff_name)
            shutil.copy(hit, dst)
            return dst
        path = orig(bir_json, tmpdir, neff_name=neff_name)
        try:
            shutil.copy(path, hit + ".tmp")
            os.replace(hit + ".tmp", hit)
        except OSError:
            pass
        return path

    b2j.compile_bir_kernel = cached
    b2j._neff_disk_cache = True


def _make_runner(nc, n_cores):
    """Jit the SPMD program once; re-invocations skip compile."""
    import jax
    from jax.sharding import Mesh, PartitionSpec
    from jax.experimental.shard_map import shard_map
    from concourse import bass2jax, mybir
    from concourse.bass2jax import _bass_exec_p, partition_id_tensor

    bass2jax.install_neuronx_cc_hook()
    _install_neff_cache()
    partition_name = nc.partition_id_tensor.name if nc.partition_id_tensor else None

    in_names, out_names, out_avals, zero_outs = [], [], [], []
    for alloc in nc.m.functions[0].allocations:
        if not isinstance(alloc, mybir.MemoryLocationSet):
            continue
        name = alloc.memorylocations[0].name
        if alloc.kind == "ExternalInput":
            if name != partition_name:
                in_names.append(name)
        elif alloc.kind == "ExternalOutput":
            out_names.append(name)
            shape = tuple(alloc.tensor_shape)
            dtype = mybir.dt.np(alloc.dtype)
            out_avals.append(jax.core.ShapedArray(shape, dtype))
            zero_outs.append(np.zeros(shape, dtype))
    n_params = len(in_names)
    n_outs = len(out_avals)
    all_in_names = list(in_names) + list(out_names)
    if partition_name is not None:
        all_in_names.append(partition_name)

    def _body(*args):
        operands = list(args)
        if partition_name is not None:
            operands.append(partition_id_tensor())
        return tuple(_bass_exec_p.bind(
            *operands,
            out_avals=tuple(out_avals),
            in_names=tuple(all_in_names),
            out_names=tuple(out_names),
            lowering_input_output_aliases=(),
            sim_require_finite=False,
            sim_require_nnan=False,
            nc=nc,
        ))

    devices = jax.devices()[:n_cores]
    mesh = Mesh(np.asarray(devices), ("core",))
    from jax.sharding import NamedSharding
    sharding = NamedSharding(mesh, PartitionSpec("core"))

    def put(arr):
        return jax.device_put(arr, sharding)

    sharded = jax.jit(
        shard_map(_body, mesh=mesh,
                  in_specs=(PartitionSpec("core"),) * (n_params + n_outs),
                  out_specs=(PartitionSpec("core"),) * n_outs,
                  check_rep=False),
        keep_unused=True,
    )

    def run(in_maps, pre=None):
        pre = pre or {}
        concat_in = []
        for i, name in enumerate(in_names):
            if name in pre:
                concat_in.append(pre[name])
                continue
            concat_in.append(np.concatenate(
                [np.asarray(m[name]) for m in in_maps], axis=0))
        concat_zeros = [
            np.zeros((n_cores * z.shape[0], *z.shape[1:]), z.dtype)
            for z in zero_outs
        ]
        out_arrs = sharded(*concat_in, *concat_zeros)
        return [
            {name: np.asarray(out_arrs[i]).reshape(n_cores, *out_avals[i].shape)[c]
             for i, name in enumerate(out_names)}
            for c in range(n_cores)
        ]

    run.put = put
    return run


def kernel(x, edge_index, edge_attr, W1, b1, W2, b2, Wf, bf):
    pre = {}

    def on_xt(XTc):
        run0 = next(iter(_prog_cache.values()), None)
        if run0 is not None:
            # start the 100MB transfer while edge prep continues on host
            pre["xt"] = run0.put(np.ascontiguousarray(
                XTc.reshape(C * P, NB * 4 * P)))

    in_maps, TPB, rowof = _host_prep(x, edge_index, edge_attr, W1, b1, W2, b2,
                                     Wf, bf, on_xt=on_xt)
    if TPB not in _prog_cache:
        nc = _build_program(TPB)
        _prog_cache[TPB] = _make_runner(nc, C)
    run = _prog_cache[TPB]
    results = run(in_maps, pre=pre)
    out = np.concatenate(
        [results[c]["out"][rowof[c, :NPC]] for c in range(C)], axis=0)
    return out.astype(np.float32)


# revision 28
# speedup vs baseline: 1.0632x; 1.0632x over previous
"""2-layer GCN (GCNConv x2 + linear) on 8 Trainium2 NeuronCores.

Full-device implementation, node-sharded across cores (12,500 dst nodes each):

  * transform  z = x @ W (PE matmuls; x shipped pre-transposed in bf16)
  * z' = dinv*z allgathered across cores (on-device AllGather collective)
  * aggregation per 64-dst scatter block: gather z'[src] rows via indirect
    DMA (128 edges / instruction), build ewn-scaled one-hot selection
    matrices from per-edge dst offsets (DVE iota+is_equal), and accumulate
    messages into PSUM with TensorE matmuls (out^T layout [feat, dst], which
    makes the next layer's matmul need no transposes).  Self-loops are
    ordinary edges with weight dinv[d]; biases are rank-1 matmuls in PSUM.

Host side does cheap O(E) integer prep: degree/norm computation, a
serpentine-dealt node permutation per core that balances edge load across
the 196 scatter bins (drops gather tiles per bin to the minimum), one
argsort of edges by (core, bin), and per-core array packing.  The x
transfer (~100 MB bf16, transposed+permuted) is started asynchronously
while edge prep continues.
"""
import numpy as np
import ml_dtypes
from contextlib import ExitStack

N = 100000
E = 3200000
IN_DIM = 512
HID = 64
OUT = 5
C = 8          # cores
P = 128
NPC = N // C   # real nodes per core (12500)
NB = 98        # 128-row transform blocks per core
SHARD = NB * P          # padded rows per core (12544)
NTOT = C * SHARD        # allgathered table rows (100352)
SB = 64                 # scatter block width (dst nodes)
NSB = SHARD // SB       # scatter blocks per core (196)
NBH = NB // 2  # xt halves (transfer pipelining)

BF16 = ml_dtypes.bfloat16

_prog_cache = {}


def _build_program(TPB, for_sim=False, no_collective=False, static_gather=False):
    """One SPMD program for all 8 cores; TPB = gather tiles per scatter block.

    for_sim=True replaces the collectives with equivalent-volume local DMA
    copies so the single-core cost simulator (TimelineSim) can run it.
    """
    from concourse import bacc, bass, tile, mybir

    NT = NSB * TPB  # gather tiles per core per layer

    nc = bacc.Bacc("TRN2", target_bir_lowering=False, debug=False,
                   num_devices=(1 if for_sim else C))
    dt = mybir.dt
    xt0_in = nc.declare_dram_parameter("xt0", [P, NBH * 4 * P], dt.bfloat16,
                                       isOutput=False)
    xt1_in = nc.declare_dram_parameter("xt1", [P, (NB - NBH) * 4 * P], dt.bfloat16,
                                       isOutput=False)
    w1_in = nc.declare_dram_parameter("w1", [P, 4 * HID], dt.bfloat16, isOutput=False)
    w2_in = nc.declare_dram_parameter("w2", [HID, HID], dt.bfloat16, isOutput=False)
    wf_in = nc.declare_dram_parameter("wf", [HID, OUT], dt.bfloat16, isOutput=False)
    b1_in = nc.declare_dram_parameter("b1", [1, HID], dt.bfloat16, isOutput=False)
    b2_in = nc.declare_dram_parameter("b2", [1, HID], dt.bfloat16, isOutput=False)
    bf_in = nc.declare_dram_parameter("bf", [1, OUT], dt.bfloat16, isOutput=False)
    dv1_in = nc.declare_dram_parameter("dv1", [P, NB], dt.float32, isOutput=False)
    src_in = nc.declare_dram_parameter("srci", [P, NT], dt.int32, isOutput=False)
    dst_in = nc.declare_dram_parameter("dsti", [P, NT], dt.int16, isOutput=False)
    ewn_in = nc.declare_dram_parameter("ewn", [P, NT], dt.bfloat16, isOutput=False)
    out_o = nc.declare_dram_parameter("out", [SHARD, OUT], dt.float32, isOutput=True)

    with tile.TileContext(nc) as tc:
        with ExitStack() as ctx:
            cpool = ctx.enter_context(tc.tile_pool(name="cpool", bufs=1))
            work = ctx.enter_context(tc.tile_pool(name="work", bufs=3))
            gpool = ctx.enter_context(tc.tile_pool(name="gpool", bufs=40))
            psum = ctx.enter_context(tc.tile_pool(name="psum", bufs=2, space="PSUM"))
            psag = ctx.enter_context(tc.tile_pool(name="psag", bufs=4, space="PSUM"))
            dram = ctx.enter_context(tc.tile_pool(name="dram", bufs=1, space="DRAM"))

            # ---- persistent constants ----
            w1_sb = cpool.tile([P, 4 * HID], dt.bfloat16)
            nc.sync.dma_start(out=w1_sb[:], in_=w1_in[:, :])
            w2_sb = cpool.tile([HID, HID], dt.bfloat16)
            nc.sync.dma_start(out=w2_sb[:], in_=w2_in[:, :])
            wf_sb = cpool.tile([HID, OUT], dt.bfloat16)
            nc.sync.dma_start(out=wf_sb[:], in_=wf_in[:, :])
            b1_sb = cpool.tile([1, HID], dt.bfloat16)
            nc.sync.dma_start(out=b1_sb[:], in_=b1_in[:, :])
            b2_sb = cpool.tile([1, HID], dt.bfloat16)
            nc.sync.dma_start(out=b2_sb[:], in_=b2_in[:, :])
            bf_sb = cpool.tile([1, OUT], dt.bfloat16)
            nc.sync.dma_start(out=bf_sb[:], in_=bf_in[:, :])
            dv1_sb = cpool.tile([P, NB], dt.float32)
            nc.sync.dma_start(out=dv1_sb[:], in_=dv1_in[:, :])
            src_sb = cpool.tile([P, NT], dt.int32)
            nc.sync.dma_start(out=src_sb[:], in_=src_in[:, :])
            dst_sb = cpool.tile([P, NT], dt.int16)
            nc.sync.dma_start(out=dst_sb[:], in_=dst_in[:, :])
            ewn_sb = cpool.tile([P, NT], dt.bfloat16)
            nc.sync.dma_start(out=ewn_sb[:], in_=ewn_in[:, :])

            iota_sb = cpool.tile([P, TPB * SB], dt.int16)
            nc.gpsimd.iota(iota_sb[:], pattern=[[0, TPB], [1, SB]], base=0,
                           channel_multiplier=0)
            ones_sb = cpool.tile([1, P], dt.bfloat16)
            nc.gpsimd.memset(ones_sb[:], 1.0)

            adsp = "Local" if (for_sim or no_collective) else "Shared"
            z1sh_d = dram.tile([SHARD, HID], dt.bfloat16)
            z1f_d = dram.tile([NTOT, HID], dt.bfloat16, addr_space=adsp)
            z2sh_d = dram.tile([SHARD, HID], dt.bfloat16)
            z2f_d = dram.tile([NTOT, HID], dt.bfloat16, addr_space=adsp)

            Relu = mybir.ActivationFunctionType.Relu
            Copy = mybir.ActivationFunctionType.Copy

            def transform1():
                for b in range(NB):
                    xtb = work.tile([P, 4 * P], dt.bfloat16, tag="xtb")
                    if b < NBH:
                        src_ap = xt0_in[:, b * 4 * P:(b + 1) * 4 * P]
                    else:
                        bb = b - NBH
                        src_ap = xt1_in[:, bb * 4 * P:(bb + 1) * 4 * P]
                    nc.sync.dma_start(out=xtb[:], in_=src_ap)
                    zp = psum.tile([P, HID], dt.float32, space="PSUM", tag="zp")
                    for k in range(4):
                        nc.tensor.matmul(
                            out=zp[:],
                            lhsT=xtb[:, k * P:(k + 1) * P],
                            rhs=w1_sb[:, k * HID:(k + 1) * HID],
                            start=(k == 0), stop=(k == 3),
                        )
                    zprime = work.tile([P, HID], dt.bfloat16, tag="zprime")
                    nc.scalar.activation(out=zprime[:], in_=zp[:], func=Copy,
                                         scale=dv1_sb[:, b:b + 1])
                    nc.sync.dma_start(out=z1sh_d[b * P:(b + 1) * P, :], in_=zprime[:])

            def transform2(h1t_sb):
                for b in range(NB):
                    zp = psum.tile([P, HID], dt.float32, space="PSUM", tag="zp")
                    nc.tensor.matmul(out=zp[:],
                                     lhsT=h1t_sb[:, b * P:(b + 1) * P],
                                     rhs=w2_sb[:, :], start=True, stop=True)
                    zprime = work.tile([P, HID], dt.bfloat16, tag="zprime")
                    nc.scalar.activation(out=zprime[:], in_=zp[:], func=Copy,
                                         scale=dv1_sb[:, b:b + 1])
                    nc.sync.dma_start(out=z2sh_d[b * P:(b + 1) * P, :], in_=zprime[:])

            def allgather(sh_d, f_d):
                if for_sim or no_collective:
                    # equivalent-volume local traffic for the cost simulator
                    for c in range(C):
                        nc.sync.dma_start(
                            out=f_d[c * SHARD:(c + 1) * SHARD, :], in_=sh_d[:, :])
                    return
                nc.gpsimd.collective_compute(
                    "AllGather", mybir.AluOpType.bypass,
                    replica_groups=[list(range(C))],
                    ins=[sh_d[:]], outs=[f_d[:]],
                )

            def aggregate(zf_d, brow_sb, ht_sb):
                """ht[f, d] = relu of sum_e ewn*z'[src] + b (self-loops are edges).

                Gathers are issued one block ahead of their consuming matmuls
                so the GPSIMD SWDGE queue (the serial resource) never idles.
                """
                def issue_gathers(b):
                    gts = []
                    for t in range(TPB):
                        gt = gpool.tile([P, HID], dt.bfloat16, tag="g")
                        ti = b * TPB + t
                        if static_gather:
                            ro = (ti % 700) * P
                            nc.sync.dma_start(out=gt[:], in_=zf_d[ro:ro + P, :])
                        else:
                            nc.gpsimd.indirect_dma_start(
                                out=gt[:],
                                out_offset=None,
                                in_=zf_d[:, :],
                                in_offset=bass.IndirectOffsetOnAxis(
                                    ap=src_sb[:, ti:ti + 1], axis=0),
                            )
                        gts.append(gt)
                    return gts

                gcur = issue_gathers(0)
                for b in range(NSB):
                    gnext = issue_gathers(b + 1) if b + 1 < NSB else None
                    oh = work.tile([P, TPB * SB], dt.bfloat16, tag="oh")
                    nc.vector.tensor_tensor(
                        out=oh[:].rearrange('p (t j) -> p t j', t=TPB),
                        in0=iota_sb[:].rearrange('p (t j) -> p t j', t=TPB),
                        in1=dst_sb[:, b * TPB:(b + 1) * TPB, None].to_broadcast(
                            [P, TPB, SB]),
                        op=mybir.AluOpType.is_equal,
                    )
                    nc.vector.tensor_tensor(
                        out=oh[:].rearrange('p (t j) -> p t j', t=TPB),
                        in0=oh[:].rearrange('p (t j) -> p t j', t=TPB),
                        in1=ewn_sb[:, b * TPB:(b + 1) * TPB, None].to_broadcast(
                            [P, TPB, SB]),
                        op=mybir.AluOpType.mult,
                    )
                    acc = psag.tile([HID, SB], dt.float32, space="PSUM", tag="acc")
                    for t in range(TPB):
                        nc.tensor.matmul(
                            out=acc[:], lhsT=gcur[t][:],
                            rhs=oh[:, t * SB:(t + 1) * SB],
                            start=(t == 0), stop=False,
                        )
                    # bias: acc[f, d] += b[f] * 1
                    nc.tensor.matmul(
                        out=acc[:], lhsT=brow_sb[0:1, :],
                        rhs=ones_sb[0:1, 0:SB], start=False, stop=True,
                    )
                    nc.scalar.activation(out=ht_sb[:, b * SB:(b + 1) * SB],
                                         in_=acc[:], func=Relu)
                    gcur = gnext

            def final(h2t_sb):
                for b in range(NB):
                    fp = psum.tile([P, OUT], dt.float32, space="PSUM", tag="fp")
                    nc.tensor.matmul(out=fp[:],
                                     lhsT=h2t_sb[:, b * P:(b + 1) * P],
                                     rhs=wf_sb[:, :], start=True, stop=False)
                    nc.tensor.matmul(out=fp[:], lhsT=ones_sb[0:1, 0:P],
                                     rhs=bf_sb[0:1, :], start=False, stop=True)
                    o_sb = work.tile([P, OUT], dt.float32, tag="osb")
                    nc.scalar.activation(out=o_sb[:], in_=fp[:], func=Copy)
                    nc.sync.dma_start(out=out_o[b * P:(b + 1) * P, :], in_=o_sb[:])

            transform1()
            allgather(z1sh_d, z1f_d)
            h1t_sb = cpool.tile([HID, SHARD], dt.bfloat16)
            aggregate(z1f_d, b1_sb, h1t_sb)
            transform2(h1t_sb)
            allgather(z2sh_d, z2f_d)
            h2t_sb = cpool.tile([HID, SHARD], dt.bfloat16)
            aggregate(z2f_d, b2_sb, h2t_sb)
            final(h2t_sb)

    nc.compile()
    return nc


def _host_prep(x, edge_index, edge_attr, W1, b1, W2, b2, Wf, bf,
               on_xt=None):
    src0 = np.asarray(edge_index[0], dtype=np.int64)
    dst0 = np.asarray(edge_index[1], dtype=np.int64)
    ew = np.asarray(edge_attr, dtype=np.float32)

    deg = np.bincount(dst0, weights=ew, minlength=N).astype(np.float32) + 1.0
    dinv = (1.0 / np.sqrt(deg)).astype(np.float32)

    # self-loops become ordinary edges: msg = dinv[d] * z'[d]
    allnodes = np.arange(N, dtype=np.int64)
    src = np.concatenate([src0, allnodes])
    dst = np.concatenate([dst0, allnodes])
    ewn = np.concatenate([(ew * dinv[dst0]).astype(np.float32), dinv])
    EE = E + N

    # per-core node permutation: serpentine-deal nodes (by degree desc) into
    # the 196 scatter bins so every bin has near-equal edge load -> TPB drops.
    degi = np.bincount(dst0, minlength=N).astype(np.int64) + 1
    rowof = np.empty((C, SHARD), dtype=np.int32)   # node-rel -> shard row
    for c in range(C):
        dg = np.zeros(SHARD, dtype=np.int64)
        dg[:NPC] = degi[c * NPC:(c + 1) * NPC]
        idx = np.argsort(-dg, kind="stable")
        rows = idx.reshape(SHARD // NSB, NSB).copy()
        rows[1::2] = rows[1::2, ::-1]
        # node at rows[i, b] -> shard row b*64 + i
        rr = np.empty(SHARD, dtype=np.int32)
        pos = (np.arange(NSB)[None, :] * SB
               + np.arange(SHARD // NSB)[:, None]).astype(np.int32)
        rr[rows.ravel()] = pos.ravel()
        rowof[c] = rr
    rowof_g = np.empty(N, dtype=np.int32)   # global node id -> table row
    for c in range(C):
        rowof_g[c * NPC:(c + 1) * NPC] = rowof[c, :NPC] + c * SHARD

    # x block-major transposed: xt[c][p, b*512 + k*128 + m] = x[row b*128+m, k*128+p]
    xb = np.asarray(x, dtype=np.float32).astype(BF16)
    xpad = np.zeros((C, SHARD, IN_DIM), dtype=BF16)
    xpad[np.arange(C)[:, None], rowof[:, :NPC]] = xb.reshape(C, NPC, IN_DIM)
    xv = xpad.reshape(C, NB, P, 4, P)
    XT0 = np.ascontiguousarray(xv[:, :NBH].transpose(0, 4, 1, 3, 2)).reshape(
        C, P, NBH * 4 * P)
    if on_xt is not None:
        on_xt("xt0", XT0)   # start the first half's transfer now
    XT1 = np.ascontiguousarray(xv[:, NBH:].transpose(0, 4, 1, 3, 2)).reshape(
        C, P, (NB - NBH) * 4 * P)
    if on_xt is not None:
        on_xt("xt1", XT1)

    core = (dst // NPC).astype(np.int32)
    nrel = (dst - core.astype(np.int64) * NPC).astype(np.int64)
    drel = rowof[core, nrel].astype(np.int64)      # permuted shard row of dst
    gblk = (core * NSB + drel // SB).astype(np.int64)  # global scatter block
    dst16 = (drel % SB).astype(np.int16)

    order = np.argsort(gblk, kind="stable")
    gsrt = gblk[order]
    cnt = np.bincount(gblk, minlength=C * NSB)
    TPB = int(np.ceil(cnt.max() / P))
    spb = TPB * P
    bstart = np.concatenate(([0], np.cumsum(cnt)))
    pos = np.arange(EE, dtype=np.int64) - bstart[gsrt]
    slot = gsrt.astype(np.int64) * spb + pos   # global slot across cores

    gidx = rowof_g[src].astype(np.int32)   # permuted row in allgathered table

    NTS = C * NSB * spb
    SRC = np.empty(NTS, dtype=np.int32)
    SRC[:] = np.resize(np.arange(SHARD, dtype=np.int32), NTS)  # spread pads
    DST = np.full(NTS, SB, dtype=np.int16)    # pad dstrel -> no one-hot match
    EWN = np.zeros(NTS, dtype=np.float32)
    SRC[slot] = gidx[order]
    DST[slot] = dst16[order]
    EWN[slot] = ewn[order]

    # device layout [C, 128, NT]: slot -> (blk, tile, p); SBUF col = blk*TPB + t
    def dev_layout(a):
        a = a.reshape(C, NSB * TPB, P)        # [(c), (blk*tile), p]
        return np.ascontiguousarray(a.transpose(0, 2, 1))

    SRCd = dev_layout(SRC)
    DSTd = dev_layout(DST)
    EWNd = dev_layout(EWN.astype(BF16))

    # dinv packs [C, 128, NB] (rows permuted like the shard)
    dpad_v = np.zeros((C, SHARD), dtype=np.float32)
    ar = np.arange(C)[:, None]
    dpad_v[ar, rowof[:, :NPC]] = dinv.reshape(C, NPC)
    DV1 = np.ascontiguousarray(dpad_v.reshape(C, NB, P).transpose(0, 2, 1))

    W1p = np.ascontiguousarray(
        np.asarray(W1, dtype=np.float32).astype(BF16)
        .reshape(4, P, HID).transpose(1, 0, 2).reshape(P, 4 * HID))
    W2p = np.asarray(W2, dtype=np.float32).astype(BF16)
    Wfp = np.asarray(Wf, dtype=np.float32).astype(BF16)
    b1p = np.asarray(b1, dtype=np.float32).astype(BF16).reshape(1, HID)
    b2p = np.asarray(b2, dtype=np.float32).astype(BF16).reshape(1, HID)
    bfp = np.asarray(bf, dtype=np.float32).astype(BF16).reshape(1, OUT)

    in_maps = []
    for c in range(C):
        in_maps.append(dict(
            xt0=XT0[c], xt1=XT1[c], w1=W1p, w2=W2p, wf=Wfp, b1=b1p, b2=b2p,
            bf=bfp, dv1=DV1[c], srci=SRCd[c], dsti=DSTd[c], ewn=EWNd[c],
        ))
    return in_maps, TPB, rowof


def _install_neff_cache():
    """Content-addressed /tmp cache around the walrus compile (the slow step)."""
    import hashlib
    import os
    import shutil
    from concourse import bass2jax as b2j

    if getattr(b2j, "_neff_disk_cache", False):
        return
    orig = b2j.compile_bir_kernel
    cache_dir = "/tmp/bass_neff_cache"
    os.makedirs(cache_dir, exist_ok=True)

    def cached(bir_json, tmpdir, neff_name="file.neff"):
        key = hashlib.sha256(bir_json).hexdigest()
        hit = os.path.join(cache_dir, key + ".neff")
        if os.path.exists(hit):
            dst = os.path.join(tmpdir, neff_name)
            shutil.copy(hit, dst)
            return dst
        path = orig(bir_json, tmpdir, neff_name=neff_name)
        try:
            shutil.copy(path, hit + ".tmp")
            os.replace(hit + ".tmp", hit)
        except OSError:
            pass
        return path

    b2j.compile_bir_kernel = cached
    b2j._neff_disk_cache = True


def _make_runner(nc, n_cores):
    """Jit the SPMD program once; re-invocations skip compile."""
    import jax
    from jax.sharding import Mesh, PartitionSpec
    from jax.experimental.shard_map import shard_map
    from concourse import bass2jax, mybir
    from concourse.bass2jax import _bass_exec_p, partition_id_tensor

    bass2jax.install_neuronx_cc_hook()
    _install_neff_cache()
    partition_name = nc.partition_id_tensor.name if nc.partition_id_tensor else None

    in_names, out_names, out_avals, zero_outs = [], [], [], []
    for alloc in nc.m.functions[0].allocations:
        if not isinstance(alloc, mybir.MemoryLocationSet):
            continue
        name = alloc.memorylocations[0].name
        if alloc.kind == "ExternalInput":
            if name != partition_name:
                in_names.append(name)
        elif alloc.kind == "ExternalOutput":
            out_names.append(name)
            shape = tuple(alloc.tensor_shape)
            dtype = mybir.dt.np(alloc.dtype)
            out_avals.append(jax.core.ShapedArray(shape, dtype))
            zero_outs.append(np.zeros(shape, dtype))
    n_params = len(in_names)
    n_outs = len(out_avals)
    all_in_names = list(in_names) + list(out_names)
    if partition_name is not None:
        all_in_names.append(partition_name)

    def _body(*args):
        operands = list(args)
        if partition_name is not None:
            operands.append(partition_id_tensor())
        return tuple(_bass_exec_p.bind(
            *operands,
            out_avals=tuple(out_avals),
            in_names=tuple(all_in_names),
            out_names=tuple(out_names),
            lowering_input_output_aliases=(),
            sim_require_finite=False,
            sim_require_nnan=False,
            nc=nc,
        ))

    devices = jax.devices()[:n_cores]
    mesh = Mesh(np.asarray(devices), ("core",))
    from jax.sharding import NamedSharding
    sharding = NamedSharding(mesh, PartitionSpec("core"))

    def put(arr):
        return jax.device_put(arr, sharding)

    sharded = jax.jit(
        shard_map(_body, mesh=mesh,
                  in_specs=(PartitionSpec("core"),) * (n_params + n_outs),
                  out_specs=(PartitionSpec("core"),) * n_outs,
                  check_rep=False),
        keep_unused=True,
    )

    def run(in_maps, pre=None):
        pre = pre or {}
        concat_in = []
        for i, name in enumerate(in_names):
            if name in pre:
                concat_in.append(pre[name])
                continue
            concat_in.append(np.concatenate(
                [np.asarray(m[name]) for m in in_maps], axis=0))
        concat_zeros = [
            np.zeros((n_cores * z.shape[0], *z.shape[1:]), z.dtype)
            for z in zero_outs
        ]
        out_arrs = sharded(*concat_in, *concat_zeros)
        return [
            {name: np.asarray(out_arrs[i]).reshape(n_cores, *out_avals[i].shape)[c]
             for i, name in enumerate(out_names)}
            for c in range(n_cores)
        ]

    run.put = put
    return run


def kernel(x, edge_index, edge_attr, W1, b1, W2, b2, Wf, bf):
    pre = {}

    def on_xt(name, arr):
        run0 = next(iter(_prog_cache.values()), None)
        if run0 is not None:
            # start each half's transfer while host packing/edge prep continues
            pre[name] = run0.put(np.ascontiguousarray(
                arr.reshape(C * P, arr.shape[2])))

    in_maps, TPB, rowof = _host_prep(x, edge_index, edge_attr, W1, b1, W2, b2,
                                     Wf, bf, on_xt=on_xt)
    if TPB not in _prog_cache:
        nc = _build_program(TPB)
        _prog_cache[TPB] = _make_runner(nc, C)
    run = _prog_cache[TPB]
    results = run(in_maps, pre=pre)
    out = np.concatenate(
        [results[c]["out"][rowof[c, :NPC]] for c in range(C)], axis=0)
    return out.astype(np.float32)
